# revision 5
# baseline (speedup 1.0000x reference)
"""Trainium2 Bass kernel for nn_ConvContract (toroidal tensor conv-contract).

The whole reference computation folds into ONE 3x3 toroidal conv with 192 input
channels -> 192 output channels:
  in-ch:  0..63 = x0[c];  64+2c+d = x1[c,:,:,d]
  out-ch: 0..63 = out0[o]; 64+2o+e = out1[o,:,:,e]
plus bias post-ops: out0 += b0; out1 += (spatial mean of out1) * b1.
On a torus the spatial mean is analytic:  mean(out1[m]) = sum_c Wsum[m,c] * mean(X[c]),
so each core only needs a 192-float AllReduce of input-channel sums (overlapped
with the matmuls) instead of a post-pass over the output.

Sharding: 8-way data parallel over image rows (32 rows/core + 1-row halo, torus
wrap done on host). Each core computes the full 192 out-channels for its slab.

Device kernel (per core), matmuls in float32r (tf32-like, ~1e-4 rel err):
  - X slab in SBUF in 4 row-segments; in-ch 128..191 duplicated into both
    partition halves so the K=64 leftover tiles pair on the two PE row-groups
    (64x128 array tiling). Concurrent row-tiles must hit different PSUM banks,
    so each out-ch group accumulates in 2 banks, combined on eviction.
  - per 2-row output chunk (512 px): 9 taps x { K=128 full-array into bank A,
    K=64 row-tile T0 -> bank A, K=64 row-tile T8 -> bank B } for out-ch 0..127
    (m01) and 128..191 (m2, weights M-padded to 128 to keep one array mode).
  - eviction (ScalarE copy + VectorE add) is bias-free so PE never waits on the
    AllReduce; the bias lands later via GpSimd tensor_scalar_add before DMA-out.
"""

import sys

sys.path.insert(0, "/opt/trn_rl_repo")

import numpy as np

import concourse.bacc as bacc
import concourse.mybir as mybir
import concourse.tile as tile
from concourse.bass_utils import run_bass_kernel_spmd

NCORES = 8
CIN = 192
COUT = 192
N = 256
ROWS = N // NCORES          # 32
HALO_ROWS = ROWS + 2        # 34
HALO_COLS = N + 2           # 258
NSEG = 4
SEG_ROWS = 10               # seg s covers slab rows 8s .. 8s+9
NCHUNK = 16                 # 2 output rows per chunk
MP = 256                    # M-padded weight columns (192 real + 64 zero)
F32 = mybir.dt.float32
F32R = mybir.dt.float32r

_COMPILED = None


def _build_nc():
    nc = bacc.Bacc("TRN2", target_bir_lowering=False, debug=False, num_devices=NCORES)

    xh = nc.dram_tensor("xh", [2, 128, HALO_ROWS, HALO_COLS], F32, kind="ExternalInput")
    wt = nc.dram_tensor("wt", [2, 128, 9, MP], F32, kind="ExternalInput")
    wsum = nc.dram_tensor("wsum", [2, 128, 128], F32, kind="ExternalInput")
    bstat = nc.dram_tensor("bstat", [64, 1], F32, kind="ExternalInput")
    out = nc.dram_tensor("out", [COUT, ROWS, N], F32, kind="ExternalOutput")

    with tile.TileContext(nc) as tc:
        with (
            tc.tile_pool(name="sbx", bufs=1) as sbx,
            tc.tile_pool(name="sbw", bufs=1) as sbw,
            tc.tile_pool(name="sbo", bufs=1) as sbo,
            tc.tile_pool(name="sbs", bufs=1) as sbs,
            tc.tile_pool(name="ps", bufs=2, space="PSUM") as ps,
            tc.tile_pool(name="dram", bufs=1, space="DRAM") as dram,
        ):
            segfree = SEG_ROWS * HALO_COLS
            XA, XB = [], []
            for s in range(NSEG):
                ta = sbx.tile([128, segfree], F32R, tag=f"xa{s}")
                tb = sbx.tile([128, segfree], F32R, tag=f"xb{s}")
                nc.sync.dma_start(
                    out=ta[:], in_=xh.ap()[0, :, 8 * s : 8 * s + SEG_ROWS, :].bitcast(F32R)
                )
                nc.sync.dma_start(
                    out=tb[:], in_=xh.ap()[1, :, 8 * s : 8 * s + SEG_ROWS, :].bitcast(F32R)
                )
                XA.append(ta)
                XB.append(tb)

            WA = sbw.tile([128, 9 * MP], F32R, tag="wa")
            WB = sbw.tile([128, 9 * MP], F32R, tag="wb")
            nc.sync.dma_start(out=WA[:], in_=wt.ap()[0].bitcast(F32R))
            nc.sync.dma_start(out=WB[:], in_=wt.ap()[1].bitcast(F32R))
            wsA = sbw.tile([128, 128], F32, tag="wsa")
            wsB = sbw.tile([128, 128], F32, tag="wsb")
            nc.sync.dma_start(out=wsA[:], in_=wsum.ap()[0])
            nc.sync.dma_start(out=wsB[:], in_=wsum.ap()[1])

            # ---- input-channel sums over this core's owned 32x256 region ----
            own = [(1, 9), (2, 8), (2, 8), (2, 7)]  # (local row0, nrows) per seg
            partA, partB = [], []
            for s in range(NSEG):
                lr0, nr = own[s]
                pa = sbs.tile([128, 1], F32, tag=f"pa{s}")
                pb = sbs.tile([64, 1], F32, tag=f"pb{s}")
                apA = XA[s][:].bitcast(F32).rearrange("p (r c) -> p r c", c=HALO_COLS)[
                    :, lr0 : lr0 + nr, 1 : 1 + N
                ]
                apB = XB[s][:].bitcast(F32).rearrange("p (r c) -> p r c", c=HALO_COLS)[
                    0:64, lr0 : lr0 + nr, 1 : 1 + N
                ]
                nc.vector.reduce_sum(pa[:], apA, axis=mybir.AxisListType.XY)
                nc.vector.reduce_sum(pb[:], apB, axis=mybir.AxisListType.XY)
                partA.append(pa)
                partB.append(pb)
            sumA = sbs.tile([128, 1], F32, tag="suma")
            sumB = sbs.tile([64, 1], F32, tag="sumb")
            nc.vector.tensor_add(sumA[:], partA[0][:], partA[1][:])
            nc.vector.tensor_add(sumA[:], sumA[:], partA[2][:])
            nc.vector.tensor_add(sumA[:], sumA[:], partA[3][:])
            nc.vector.tensor_add(sumB[:], partB[0][:], partB[1][:])
            nc.vector.tensor_add(sumB[:], sumB[:], partB[2][:])
            nc.vector.tensor_add(sumB[:], sumB[:], partB[3][:])

            cin = dram.tile([CIN, 1], F32)
            ccout = dram.tile([CIN, 1], F32)
            nc.sync.dma_start(out=cin[0:128, :], in_=sumA[:])
            nc.sync.dma_start(out=cin[128:192, :], in_=sumB[:])
            nc.gpsimd.collective_compute(
                "AllReduce",
                mybir.AluOpType.add,
                replica_groups=[list(range(NCORES))],
                ins=[cin.opt()],
                outs=[ccout.opt()],
            )
            rA = sbs.tile([128, 1], F32, tag="ra")
            rB = sbs.tile([64, 1], F32, tag="rb")
            nc.sync.dma_start(out=rA[:], in_=ccout[0:128, :])
            nc.sync.dma_start(out=rB[:], in_=ccout[128:192, :])

            # matvec -> dyn bias for out-ch 64..191 (reuses an m01 PSUM slot)
            pmv = ps.tile([128, 1], F32, tag="m01")
            nc.tensor.matmul(pmv[:], wsA[:], rA[:], start=True, stop=False)
            nc.tensor.matmul(pmv[:], wsB[0:64, :], rB[:], start=False, stop=True)
            dyn = sbs.tile([128, 1], F32, tag="dyn")
            nc.vector.tensor_copy(dyn[:], pmv[:])
            biasAB = sbs.tile([128, 1], F32, tag="biasab")
            biasC = sbs.tile([64, 1], F32, tag="biasc")
            nc.sync.dma_start(out=biasAB[0:64, :], in_=bstat.ap())
            nc.sync.dma_start(out=biasAB[64:128, :], in_=dyn[0:64, :])
            nc.sync.dma_start(out=biasC[:], in_=dyn[64:128, :])

            # ---- main conv loop ----
            OA = sbo.tile([128, NCHUNK * 512], F32, tag="oa")
            OB = sbo.tile([64, NCHUNK * 512], F32, tag="ob")
            WA_r = WA[:].rearrange("p (t m) -> p t m", t=9)
            WB_r = WB[:].rearrange("p (t m) -> p t m", t=9)
            taps = [(ky, kx) for ky in range(3) for kx in range(3)]

            for r in range(NCHUNK):
                s = r // 4
                lrb = 2 * r - 8 * s
                XAr = XA[s][:].rearrange("p (r c) -> p r c", c=HALO_COLS)
                XBr = XB[s][:].rearrange("p (r c) -> p r c", c=HALO_COLS)

                pm = ps.tile([128, 512], F32, tag="m01")
                pm_b = ps.tile([128, 512], F32, tag="m01b")
                p2 = ps.tile([128, 512], F32, tag="m2")
                p2_b = ps.tile([128, 512], F32, tag="m2b")

                def rhsA(ky, kx):
                    return XAr[:, lrb + ky : lrb + ky + 2, kx : kx + N]

                def rhsB(h, ky, kx):
                    return XBr[
                        64 * h : 64 * h + 64, lrb + ky : lrb + ky + 2, kx : kx + N
                    ]

                # phase 1: K=128 full-array tiles (in-ch 0..127), 9 taps x {m01, m2}
                for t, (ky, kx) in enumerate(taps):
                    nc.tensor.matmul(
                        pm[:], WA_r[:, t, 0:128], rhsA(ky, kx),
                        start=(t == 0), stop=False,
                    )
                    nc.tensor.matmul(
                        p2[:], WA_r[:, t, 128:256], rhsA(ky, kx),
                        start=(t == 0), stop=False,
                    )
                # phase 2: K=64 row-tiles (in-ch 128..191); T0 (partitions 0..63)
                # accumulates into pm/p2, T8 (64..127) into pm_b/p2_b.
                for t, (ky, kx) in enumerate(taps):
                    h1 = t % 2        # m01 row-half
                    h2 = 1 - h1       # m2 row-half (balances T0/T8 load)
                    nc.tensor.matmul(
                        (pm if h1 == 0 else pm_b)[:],
                        WB_r[64 * h1 : 64 * h1 + 64, t, 0:128],
                        rhsB(h1, ky, kx),
                        start=(t == 1 and h1 == 1),
                        stop=(t == 8 if h1 == 0 else t == 7),
                    )
                    nc.tensor.matmul(
                        (p2 if h2 == 0 else p2_b)[:],
                        WB_r[64 * h2 : 64 * h2 + 64, t, 128:256],
                        rhsB(h2, ky, kx),
                        start=(t == 0 and h2 == 1),
                        stop=(t == 7 if h2 == 0 else t == 8),
                    )

                # bias-free eviction: ScalarE copies bank A, VectorE adds bank B
                oa = OA[:, r * 512 : (r + 1) * 512]
                ob = OB[:, r * 512 : (r + 1) * 512]
                nc.scalar.activation(oa, pm[:], mybir.ActivationFunctionType.Copy)
                nc.vector.tensor_add(oa, oa, pm_b[:])
                nc.scalar.activation(ob, p2[0:64, :], mybir.ActivationFunctionType.Copy)
                nc.vector.tensor_add(ob, ob, p2_b[0:64, :])

                # late bias on GpSimd (waits for the AllReduce path, not the PE)
                nc.gpsimd.tensor_scalar_add(oa, oa, biasAB[:, 0:1])
                nc.gpsimd.tensor_scalar_add(ob, ob, biasC[:, 0:1])
                nc.sync.dma_start(out=out.ap()[0:128, 2 * r : 2 * r + 2, :], in_=oa)
                nc.sync.dma_start(out=out.ap()[128:192, 2 * r : 2 * r + 2, :], in_=ob)

    nc.compile()
    return nc


def _host_prep(x0, x1, fil0, fil1, fil2, w00, w01, w10, w11, b0, b1):
    f32 = np.float32
    syn = lambda w, f: np.einsum("oik,k...->oi...", w, f).astype(f32)
    s00 = syn(w00, fil0)
    s01 = syn(w01, fil1)
    s10 = syn(w10, fil1)
    s11 = syn(w11, fil2)

    W = np.zeros((COUT, CIN, 3, 3), f32)
    W[:64, :64] = s00
    W[:64, 64:] = s10.transpose(0, 1, 4, 2, 3).reshape(64, 128, 3, 3)
    W[64:, :64] = s01.transpose(0, 4, 1, 2, 3).reshape(128, 64, 3, 3)
    W[64:, 64:] = s11.transpose(0, 5, 1, 4, 2, 3).reshape(128, 128, 3, 3)

    # wt[g, p, tap, m] = W[m, k(g,p), ky, kx], M padded 192 -> 256 with zeros
    Wk = np.ascontiguousarray(W.transpose(1, 2, 3, 0)).reshape(CIN, 9, COUT)
    wt = np.zeros((2, 128, 9, MP), f32)
    wt[0, :, :, :COUT] = Wk[:128]
    wt[1, :64, :, :COUT] = Wk[128:]
    wt[1, 64:, :, :COUT] = Wk[128:]

    S = W.sum(axis=(2, 3))
    b1f = np.repeat(b1.reshape(64), 2)
    wsum_f = (S[64:] * b1f[:, None] / (N * N)).T.astype(f32)  # (c, m)
    wsum = np.zeros((2, 128, 128), f32)
    wsum[0] = wsum_f[:128]
    wsum[1, :64] = wsum_f[128:]

    bstat = np.ascontiguousarray(b0.reshape(64, 1), dtype=f32)

    Xf = np.empty((CIN, N, N), f32)
    Xf[:64] = x0
    Xf[64:] = x1.transpose(0, 3, 1, 2).reshape(128, N, N)
    cols = np.arange(-1, N + 1) % N
    Xc = np.ascontiguousarray(Xf[:, :, cols])

    in_maps = []
    for r in range(NCORES):
        rows = np.arange(ROWS * r - 1, ROWS * r + ROWS + 1) % N
        slab = Xc[:, rows, :]
        xhs = np.empty((2, 128, HALO_ROWS, HALO_COLS), f32)
        xhs[0] = slab[:128]
        xhs[1, :64] = slab[128:]
        xhs[1, 64:] = slab[128:]
        in_maps.append({"xh": xhs, "wt": wt, "wsum": wsum, "bstat": bstat})
    return in_maps


def kernel(x0, x1, fil0, fil1, fil2, w00, w01, w10, w11, b0, b1):
    global _COMPILED
    if _COMPILED is None:
        _COMPILED = _build_nc()
    nc = _COMPILED

    in_maps = _host_prep(
        np.asarray(x0, np.float32), np.asarray(x1, np.float32),
        np.asarray(fil0, np.float32), np.asarray(fil1, np.float32),
        np.asarray(fil2, np.float32),
        np.asarray(w00, np.float32), np.asarray(w01, np.float32),
        np.asarray(w10, np.float32), np.asarray(w11, np.float32),
        np.asarray(b0, np.float32), np.asarray(b1, np.float32),
    )
    res = run_bass_kernel_spmd(nc, in_maps, core_ids=list(range(NCORES)))

    full = np.empty((COUT, N, N), np.float32)
    for r in range(NCORES):
        full[:, ROWS * r : ROWS * (r + 1), :] = res.results[r]["out"]
    out0 = full[:64]
    out1 = np.ascontiguousarray(full[64:].reshape(64, 2, N, N).transpose(0, 2, 3, 1))
    return (out0, out1)
